# revision 21
# baseline (speedup 1.0000x reference)
"""GATv2 encoder (3-layer, 8-head) Trainium2 Bass kernel, node-sharded across
8 NeuronCores.

Strategy:
  - Nodes are partitioned contiguously across 8 cores (1250 each). All
    per-node tensors (h, xl, xr, conv outputs, LayerNorm) live on the owning
    core; weights are replicated.
  - Edges (incl. self loops) are bucketed by destination core/row-block on the
    host. Each core handles the incoming edges of its own nodes.
  - Per layer: each core computes its shard of the source/target transforms
    (xl = h@Wl, xr = h@Wr) with PE matmuls, then an AllGather makes the full
    source transform xl available everywhere (bf16). Cross-partition source
    rows are fetched with indirect-DMA gathers.
  - Segment softmax over incoming edges is computed without the segment-max
    pass (logits are O(1) so exp is safe, and softmax is shift-invariant).
    The leaky-relu is decomposed as lrelu(z) = 0.8*relu(z) + 0.2*z, and the
    per-edge attention dot att.z splits into att.xl[src] + att.xr[dst]; the
    dst part is constant within a softmax segment and is dropped, while the
    src part (a per-node 8-vector) is appended to the gathered rows.
  - Segment sums (softmax denominators and the weighted feature aggregation)
    are PE matmuls against host-built 0/1 edge->dst selection matrices; the
    dst->edge broadcast of xr uses their transpose.

Host-side work is limited to integer index/bucketing, weight dtype casts, and
constant folding of biases (all numpy)."""

import sys
sys.path.insert(0, "/opt/trn_rl_repo")

import numpy as np
import ml_dtypes

import os

import concourse.bass as bass
import concourse.bacc as bacc
import concourse.mybir as mybir
import concourse.tile as tile
from concourse.bass_utils import run_bass_kernel_spmd

_STOP_AFTER = os.environ.get("GAT_STOP_AFTER", "")

BF = ml_dtypes.bfloat16
F32 = mybir.dt.float32
BF16 = mybir.dt.bfloat16
I32 = mybir.dt.int32
AF = mybir.ActivationFunctionType
OP = mybir.AluOpType

P = 128
NCORES = 8
EPS = 1e-5
SLOPE = 0.2


# ----------------------------------------------------------------- host prep
def _edge_buckets(src, dst, n, sh, nb):
    """Bucket edges by (dst core, dst 128-row block); pad each bucket to a
    uniform tile count T. Returns idx [NC,NB,128,T] i32, sem [NC,NB,128,T,128]
    (edge-major, e on axis-2 partitions), sdt [NC,NB,128,T,128] (dst-major),
    and T."""
    order = np.argsort(dst, kind="stable")
    src_s = src[order].astype(np.int64)
    dst_s = dst[order].astype(np.int64)
    core = dst_s // sh
    loc = dst_s - core * sh
    blk = loc // P
    dloc = loc % P
    seg = core * nb + blk
    counts = np.bincount(seg, minlength=NCORES * nb)
    T = max(1, int(np.ceil(counts.max() / P)))
    starts = np.zeros(NCORES * nb + 1, np.int64)
    starts[1:] = np.cumsum(counts)
    eoff = np.arange(len(dst_s)) - starts[seg]
    t_slot = (eoff // P).astype(np.int64)
    p_slot = (eoff % P).astype(np.int64)

    idx = np.zeros((NCORES, nb, P, T), np.int32)
    idx[core, blk, p_slot, t_slot] = src_s
    sem = np.zeros((NCORES, nb, P, T, P), BF)   # [.., e_p, t, d]
    sem[core, blk, p_slot, t_slot, dloc] = 1
    sdt = np.zeros((NCORES, nb, P, T, P), BF)   # [.., d_p, t, e]
    sdt[core, blk, dloc, t_slot, p_slot] = 1
    return idx, sem, sdt, T


def _prep(inputs, n, din, h, c, out_w):
    hd = h * c
    sh = n // NCORES
    nb = (sh + P - 1) // P
    shp = nb * P
    f = lambda a: np.asarray(a, np.float32)

    x = f(inputs["x"])
    ei = np.asarray(inputs["edge_index"])
    src = np.concatenate([ei[0], np.arange(n, dtype=ei.dtype)])
    dst = np.concatenate([ei[1], np.arange(n, dtype=ei.dtype)])
    idx, sem, sdt, T = _edge_buckets(src, dst, n, sh, nb)

    W_in, b_in = f(inputs["W_in"]), f(inputs["b_in"])
    w = {}
    w["T"], w["sh"], w["nb"], w["shp"] = T, sh, nb, shp
    w["win"] = W_in.astype(BF)
    w["bin"] = b_in.reshape(1, hd)

    att = [f(inputs["att0"]), f(inputs["att1"]), f(inputs["att2"])]
    for l, wd in ((0, hd), (1, hd), (2, out_w)):
        Wl, bl = f(inputs[f"Wl{l}"]), f(inputs[f"bl{l}"])
        Wr, br = f(inputs[f"Wr{l}"]), f(inputs[f"br{l}"])
        bc = f(inputs[f"bc{l}"])
        a = att[l]                      # [heads, chan]
        heads = a.shape[0]
        w[f"wl{l}"] = Wl.astype(BF)
        w[f"wr{l}"] = Wr.astype(BF)
        # per-node attention-dot projection, pre-scaled by SLOPE:
        # wla[k, head] = SLOPE * sum_c Wl[k, (head,c)] * a[head, c]
        wla = SLOPE * np.einsum("khc,hc->kh", Wl.reshape(hd, heads, a.shape[1]), a)
        wla8 = np.zeros((hd, 8), np.float32)
        wla8[:, :heads] = wla
        w[f"wla{l}"] = wla8.astype(BF)
        bz = (bl + br).reshape(1, wd)
        bo = (bl + bc).reshape(1, wd)
        if l == 0:
            # layer-0 input is h0_stored = elu(..)+1 -> subtract column sums
            bz = bz - (Wl.sum(0) + Wr.sum(0)).reshape(1, wd)
            bo = bo - Wl.sum(0).reshape(1, wd)
        # bo is folded into xl (sum(alpha)=1 carries it through the segment
        # softmax exactly); compensate in the logit bias which sees xl+bo
        bz = bz - bo
        w[f"bz{l}"] = bz
        w[f"bo{l}"] = bo
        # 0.8-scaled replicated attention vector [128, wd]
        ar = np.broadcast_to((0.8 * a.reshape(1, -1)), (P, wd)).astype(BF)
        w[f"attr{l}"] = np.ascontiguousarray(ar)
        g_ = f(inputs[f"g{l}"])
        be_ = f(inputs[f"be{l}"])
        w[f"gtriv{l}"] = bool(np.allclose(g_, 1) and np.allclose(be_, 0))
        w[f"grep{l}"] = np.ascontiguousarray(
            np.broadcast_to(g_.reshape(1, wd), (P, wd))).astype(BF)
        w[f"berep{l}"] = np.ascontiguousarray(
            np.broadcast_to(be_.reshape(1, wd), (P, wd))).astype(BF)
    w["wres2"] = f(inputs["Wres2"]).astype(BF)
    w["bres2"] = f(inputs["bres2"]).reshape(1, out_w)

    padv = np.zeros((P, nb), np.float32)
    for b in range(nb):
        rows = np.arange(b * P, (b + 1) * P)
        padv[rows >= sh, b] = 1.0
    w["padv"] = padv
    w["ident"] = np.eye(P, dtype=np.float32).astype(BF)
    w["ones1"] = np.ones((1, P), np.float32)

    # per-core tensors
    cores = []
    for k in range(NCORES):
        xs = np.zeros((din, shp), np.float32)
        xs[:, :sh] = x[k * sh:(k + 1) * sh].T
        cores.append({
            "xT": xs.astype(BF),
            "idx": idx[k],
            "sem": np.ascontiguousarray(sem[k]),
            "sdt": np.ascontiguousarray(sdt[k]),
        })
    return w, cores


# ------------------------------------------------------------- device program
def _build(wc, n, din, h, c, out_w):
    hd = h * c
    T, sh, nb, shp = wc["T"], wc["sh"], wc["nb"], wc["shp"]

    nc = bacc.Bacc(trn_type="TRN2", num_devices=NCORES)
    DT = lambda nm, shv, dt: nc.dram_tensor(nm, shv, dt, kind="ExternalInput")

    xT_d = DT("xT", [din, shp], BF16)
    idx_d = DT("idx", [nb, P, T], I32)
    sem_d = DT("sem", [nb, P, T, P], BF16)
    sdt_d = DT("sdt", [nb, P, T, P], BF16)
    padv_d = DT("padv", [P, nb], F32)
    ident_d = DT("ident", [P, P], BF16)
    ones1_d = DT("ones1", [1, P], F32)
    win_d = DT("win", [din, hd], BF16)
    bin_d = DT("bin", [1, hd], F32)
    wl_d, wr_d, wla_d, bz_d, bo_d, attr_d = {}, {}, {}, {}, {}, {}
    grep_d, berep_d = {}, {}
    for l, wd in ((0, hd), (1, hd), (2, out_w)):
        wl_d[l] = DT(f"wl{l}", [hd, wd], BF16)
        wr_d[l] = DT(f"wr{l}", [hd, wd], BF16)
        wla_d[l] = DT(f"wla{l}", [hd, 8], BF16)
        bz_d[l] = DT(f"bz{l}", [1, wd], F32)
        bo_d[l] = DT(f"bo{l}", [1, wd], F32)
        attr_d[l] = DT(f"attr{l}", [P, wd], BF16)
        if not wc[f"gtriv{l}"]:
            grep_d[l] = DT(f"grep{l}", [P, wd], BF16)
            berep_d[l] = DT(f"berep{l}", [P, wd], BF16)
    wres_d = DT("wres2", [hd, out_w], BF16)
    bres_d = DT("bres2", [1, out_w], F32)

    out_d = nc.dram_tensor("out", [sh, out_w], F32, kind="ExternalOutput")

    # internal DRAM
    h_i = [nc.dram_tensor(f"h{l}", [shp, hd], F32) for l in range(3)]
    hb_i = [nc.dram_tensor(f"hb{l}", [shp, hd], BF16) for l in range(3)]
    wd_l = {0: hd, 1: hd, 2: out_w}
    tl_l = {0: 8, 1: 8, 2: 32}
    xl_i = {l: nc.dram_tensor(f"xlsh{l}", [shp, wd_l[l] + tl_l[l]], BF16)
            for l in wd_l}
    xr_i = {l: nc.dram_tensor(f"xrsh{l}", [shp, wd_l[l]], BF16) for l in wd_l}
    ag_i = {l: nc.dram_tensor(f"ag{l}", [n, wd_l[l] + tl_l[l]], BF16,
                              addr_space="Shared") for l in wd_l}
    res2_i = nc.dram_tensor("res2", [shp, out_w], F32)

    with tile.TileContext(nc) as tc:
        _emit(nc, tc, locals(), wc, n, din, h, c, out_w)
    nc.compile()
    return nc


def _emit(nc, tc, tn, wc, n, din, h, c, out_w):
    hd = h * c
    T, sh, nb, shp = wc["T"], wc["sh"], wc["nb"], wc["shp"]
    wd_l = {0: hd, 1: hd, 2: out_w}
    h_i, hb_i, xl_i, xr_i, ag_i, res2_i = (tn["h_i"], tn["hb_i"], tn["xl_i"],
                                           tn["xr_i"], tn["ag_i"], tn["res2_i"])
    out_d = tn["out_d"]

    # ---- constants pool (resident) --------------------------------------
    with tc.tile_pool(name="const", bufs=1) as cp:
        _emit_body(nc, tc, tn, wc, n, din, h, c, out_w, cp)


def _emit_body(nc, tc, tn, wc, n, din, h, c, out_w, cp):
    hd = h * c
    T, sh, nb, shp = wc["T"], wc["sh"], wc["nb"], wc["shp"]
    wd_l = {0: hd, 1: hd, 2: out_w}
    tl_l = {0: 8, 1: 8, 2: 32}
    h_i, hb_i, xl_i, xr_i, ag_i, res2_i = (tn["h_i"], tn["hb_i"], tn["xl_i"],
                                           tn["xr_i"], tn["ag_i"], tn["res2_i"])
    out_d = tn["out_d"]
    ident_s = cp.tile([P, P], BF16, tag="ident")
    nc.sync.dma_start(ident_s[:], tn["ident_d"][:])
    ones1_s = cp.tile([1, P], F32, tag="ones1")
    nc.sync.dma_start(ones1_s[:], tn["ones1_d"][:])
    padv_s = cp.tile([P, nb], F32, tag="padv")
    nc.sync.dma_start(padv_s[:], tn["padv_d"][:])
    eps_s = cp.tile([P, 1], F32, tag="eps")
    nc.vector.memset(eps_s[:], EPS)
    zero_s = cp.tile([P, hd], F32, tag="zeros")
    nc.vector.memset(zero_s[:], 0.0)
    zero_b = cp.tile([P, hd + 32], BF16, tag="zerosb")
    nc.vector.memset(zero_b[:], 0.0)

    # zero the padded tail rows of node-sharded DRAM buffers
    pad0 = sh - (nb - 1) * P          # valid rows in last block
    npad = shp - sh
    if npad > 0:
        for buf in h_i:
            nc.sync.dma_start(buf[sh:shp, :], zero_s[:npad, :])
        for buf in hb_i:
            nc.sync.dma_start(buf[sh:shp, :], zero_b[:npad, :hd])
        for l in wd_l:
            nc.sync.dma_start(xl_i[l][sh:shp, :],
                              zero_b[:npad, :wd_l[l] + tl_l[l]])
            nc.sync.dma_start(xr_i[l][sh:shp, :], zero_b[:npad, :wd_l[l]])
        nc.sync.dma_start(res2_i[sh:shp, :], zero_s[:npad, :out_w])

    def _stopped(tag):
        if _STOP_AFTER and tag == _STOP_AFTER:
            zo = cp.tile([P, out_w], F32, tag="zout")
            nc.vector.memset(zo[:], 0.0)
            for b in range(nb):
                rows = min(P, sh - b * P)
                nc.sync.dma_start(out_d[b * P:b * P + rows, :], zo[:rows, :])
            return True
        return False

    # ---- input MLP: h0 = elu(x @ W_in + b_in) + 1 ------------------------
    with (tc.tile_pool(name="in_sb", bufs=1) as sp,
          tc.tile_pool(name="in_sb2", bufs=2) as sp2,
          tc.tile_pool(name="in_ps", bufs=2, space="PSUM") as pp):
        kin = din // P
        xt_s = sp.tile([P, kin, shp], BF16, tag="xt")
        nc.sync.dma_start(xt_s[:], tn["xT_d"][:].rearrange("(k p) n -> p k n", p=P))
        win_s = sp.tile([P, kin, hd], BF16, tag="win")
        nc.sync.dma_start(win_s[:], tn["win_d"][:].rearrange("(k p) f -> p k f", p=P))
        bin_s = sp.tile([1, hd], F32, tag="bin")
        nc.sync.dma_start(bin_s[:], tn["bin_d"][:])
        for b in range(nb):
            hsb = sp2.tile([P, hd], F32, tag="hsb")
            for fc in range(hd // 512):
                ps = pp.tile([P, 512], F32, tag="ps")
                for k in range(kin):
                    nc.tensor.matmul(ps[:], lhsT=xt_s[:, k, b * P:(b + 1) * P],
                                     rhs=win_s[:, k, fc * 512:(fc + 1) * 512],
                                     start=(k == 0), stop=False)
                nc.tensor.matmul(ps[:], lhsT=ones1_s[:],
                                 rhs=bin_s[:, fc * 512:(fc + 1) * 512],
                                 start=False, stop=True)
                sl_ = slice(fc * 512, (fc + 1) * 512)
                mi = sp2.tile([P, 512], F32, tag="mi")
                nc.vector.tensor_scalar_min(mi[:], ps[:], 0.0)
                u = sp2.tile([P, 512], F32, tag="u")
                nc.scalar.activation(u[:], mi[:], AF.Exp)
                nc.vector.tensor_scalar_max(hsb[:, sl_], ps[:], 0.0)
                nc.vector.tensor_add(hsb[:, sl_], hsb[:, sl_], u[:])
            hbb = sp2.tile([P, hd], BF16, tag="hbb")
            nc.vector.tensor_copy(hbb[:], hsb[:])
            rows = min(P, sh - b * P)
            nc.sync.dma_start(h_i[0][b * P:b * P + rows, :], hsb[:rows, :])
            nc.sync.dma_start(hb_i[0][b * P:b * P + rows, :], hbb[:rows, :])

    if _stopped("in"):
        return
    # ---- per-layer ------------------------------------------------------
    for l in range(3):
        wd = wd_l[l]
        nfc = max(1, wd // 512)
        fcw = min(wd, 512)
        # -- matmul phase: xl(+sl), xr(+bz), l2: res2 ---------------------
        with (tc.tile_pool(name=f"m{l}_sb", bufs=1) as sp,
              tc.tile_pool(name=f"m{l}_sb2", bufs=3) as sp2,
              tc.tile_pool(name=f"m{l}_ps", bufs=2, space="PSUM") as pp):
            kk = hd // P
            ht_s = sp.tile([P, kk, shp], BF16, tag="ht")
            for k in range(kk):
                nc.sync.dma_start_transpose(ht_s[:, k, :],
                                            hb_i[l][:, k * P:(k + 1) * P])
            wl_s = sp.tile([P, kk, wd], BF16, tag="wl")
            nc.sync.dma_start(wl_s[:], tn["wl_d"][l][:].rearrange(
                "(k p) f -> p k f", p=P))
            wr_s = sp.tile([P, kk, wd], BF16, tag="wr")
            nc.sync.dma_start(wr_s[:], tn["wr_d"][l][:].rearrange(
                "(k p) f -> p k f", p=P))
            wla_s = sp.tile([P, kk, 8], BF16, tag="wla")
            nc.sync.dma_start(wla_s[:], tn["wla_d"][l][:].rearrange(
                "(k p) f -> p k f", p=P))
            bz_s = sp.tile([1, wd], F32, tag="bz")
            nc.sync.dma_start(bz_s[:], tn["bz_d"][l][:])
            bom_s = sp.tile([1, wd], F32, tag="bom")
            nc.sync.dma_start(bom_s[:], tn["bo_d"][l][:])
            if l == 2:
                wres_s = sp.tile([P, kk, out_w], BF16, tag="wres")
                nc.sync.dma_start(wres_s[:], tn["wres_d"][:].rearrange(
                    "(k p) f -> p k f", p=P))
                bres_s = sp.tile([1, out_w], F32, tag="bres")
                nc.sync.dma_start(bres_s[:], tn["bres_d"][:])
            for b in range(nb):
                bsl = slice(b * P, (b + 1) * P)
                xl_sb = sp2.tile([P, wd + tl_l[l]], BF16, tag="xl_sb")
                if tl_l[l] > 8:
                    nc.vector.memset(xl_sb[:, wd + 8:], 0.0)
                for fc in range(nfc):
                    fsl = slice(fc * fcw, (fc + 1) * fcw)
                    ps = pp.tile([P, fcw], F32, tag="psm")
                    for k in range(kk):
                        nc.tensor.matmul(ps[:], lhsT=ht_s[:, k, bsl],
                                         rhs=wl_s[:, k, fsl],
                                         start=(k == 0), stop=False)
                    nc.tensor.matmul(ps[:], lhsT=ones1_s[:], rhs=bom_s[:, fsl],
                                     start=False, stop=True)
                    nc.vector.tensor_copy(xl_sb[:, fsl], ps[:])
                psl = pp.tile([P, 8], F32, tag="psl")
                for k in range(kk):
                    nc.tensor.matmul(psl[:], lhsT=ht_s[:, k, bsl],
                                     rhs=wla_s[:, k, :],
                                     start=(k == 0), stop=(k == kk - 1))
                nc.vector.tensor_copy(xl_sb[:, wd:wd + 8], psl[:])
                nc.sync.dma_start(xl_i[l][bsl, :], xl_sb[:])
                xr_sb = sp2.tile([P, wd], BF16, tag="xr_sb")
                for fc in range(nfc):
                    fsl = slice(fc * fcw, (fc + 1) * fcw)
                    ps = pp.tile([P, fcw], F32, tag="psm")
                    for k in range(kk):
                        nc.tensor.matmul(ps[:], lhsT=ht_s[:, k, bsl],
                                         rhs=wr_s[:, k, fsl],
                                         start=(k == 0), stop=False)
                    nc.tensor.matmul(ps[:], lhsT=ones1_s[:], rhs=bz_s[:, fsl],
                                     start=False, stop=True)
                    nc.vector.tensor_copy(xr_sb[:, fsl], ps[:])
                nc.sync.dma_start(xr_i[l][bsl, :], xr_sb[:])
                if l == 2:
                    ps = pp.tile([P, out_w], F32, tag="psm")
                    for k in range(kk):
                        nc.tensor.matmul(ps[:], lhsT=ht_s[:, k, bsl],
                                         rhs=wres_s[:, k, :],
                                         start=(k == 0), stop=False)
                    nc.tensor.matmul(ps[:], lhsT=ones1_s[:], rhs=bres_s[:],
                                     start=False, stop=True)
                    rsb = sp2.tile([P, out_w], F32, tag="rsb")
                    nc.vector.tensor_copy(rsb[:], ps[:])
                    nc.sync.dma_start(res2_i[bsl, :], rsb[:])

        if _stopped(f"m{l}"):
            return
        # -- AllGather of xl shard ---------------------------------------
        nc.gpsimd.collective_compute(
            "AllGather", OP.bypass,
            replica_groups=[list(range(NCORES))],
            ins=[xl_i[l][:sh, :]], outs=[ag_i[l][:]],
        )

        if _stopped(f"ag{l}"):
            return
        # -- edge phase ---------------------------------------------------
        heads = h if l < 2 else 1
        cw = c if l < 2 else out_w
        ew = 8 if l < 2 else 1          # e-slot width per tile
        with (tc.tile_pool(name=f"e{l}_sb", bufs=1) as sp,
              tc.tile_pool(name=f"e{l}_g", bufs=2) as gp,
              tc.tile_pool(name=f"e{l}_sb2", bufs=3) as sp2,
              tc.tile_pool(name=f"e{l}_ep", bufs=2) as ep,
              tc.tile_pool(name=f"e{l}_zp", bufs=2, space="PSUM") as zpp,
              tc.tile_pool(name=f"e{l}_op", bufs=1, space="PSUM") as opp):
            attr_s = sp.tile([P, wd], BF16, tag="attr")
            nc.sync.dma_start(attr_s[:], tn["attr_d"][l][:])
            if not wc[f"gtriv{l}"]:
                g_s = sp.tile([P, wd], BF16, tag="grep")
                nc.sync.dma_start(g_s[:], tn["grep_d"][l][:])
                be_s = sp.tile([P, wd], BF16, tag="berep")
                nc.sync.dma_start(be_s[:], tn["berep_d"][l][:])
            for b in range(nb):
                bsl = slice(b * P, (b + 1) * P)
                idx_s = gp.tile([P, T], I32, tag="idx")
                nc.sync.dma_start(idx_s[:], tn["idx_d"][b, :, :])
                g = gp.tile([P, T, wd + tl_l[l]], BF16, tag="g")
                for t in range(T):
                    nc.gpsimd.indirect_dma_start(
                        out=g[:, t, :], out_offset=None, in_=ag_i[l][:],
                        in_offset=bass.IndirectOffsetOnAxis(
                            ap=idx_s[:, t:t + 1], axis=0))
                sem_s = gp.tile([P, T, P], BF16, tag="sem")
                nc.sync.dma_start(sem_s[:], tn["sem_d"][b, :, :, :])
                sdt_s = gp.tile([P, T, P], BF16, tag="sdt")
                nc.sync.dma_start(sdt_s[:], tn["sdt_d"][b, :, :, :])
                xr_s = gp.tile([P, wd], BF16, tag="xr")
                nc.sync.dma_start(xr_s[:], xr_i[l][bsl, :])

                e_blk = ep.tile([P, T * ew], F32, tag="e_blk")
                for t in range(T):
                    zp = zpp.tile([P, wd], F32, tag="zp")
                    for fc in range(nfc):
                        fsl = slice(fc * fcw, (fc + 1) * fcw)
                        nc.tensor.matmul(zp[:, fsl], lhsT=sdt_s[:, t, :],
                                         rhs=xr_s[:, fsl], start=True, stop=False)
                        nc.tensor.matmul(zp[:, fsl], lhsT=ident_s[:],
                                         rhs=g[:, t, fsl], start=False, stop=True)
                    r = sp2.tile([P, wd], BF16, tag="r")
                    nc.scalar.activation(r[:], zp[:], AF.Relu)
                    m = sp2.tile([P, wd], BF16, tag="m")
                    nc.vector.tensor_mul(m[:], r[:], attr_s[:])
                    esl = e_blk[:, t * ew:(t + 1) * ew]
                    nc.vector.tensor_reduce(
                        esl, m[:].rearrange("p (h c) -> p h c", h=heads),
                        axis=mybir.AxisListType.X, op=OP.add)
                # add gathered 0.2*att.xl[src] tail, exponentiate
                tail = g[:, :, wd:wd + ew]
                nc.vector.tensor_tensor(out=e_blk[:].rearrange(
                    "p (t w) -> p t w", t=T), in0=e_blk[:].rearrange(
                    "p (t w) -> p t w", t=T), in1=tail, op=OP.add)
                nc.scalar.activation(e_blk[:], e_blk[:], AF.Exp)
                exb = ep.tile([P, T * ew], BF16, tag="exb")
                nc.vector.tensor_copy(exb[:], e_blk[:])

                op_ = opp.tile([P, wd], F32, tag="op")
                dp = opp.tile([P, 8], F32, tag="dp")
                for t in range(T):
                    v = sp2.tile([P, wd], BF16, tag="v")
                    if l < 2:
                        nc.vector.tensor_tensor(
                            out=v[:].rearrange("p (h c) -> p h c", h=heads),
                            in0=g[:, t, :wd].rearrange("p (h c) -> p h c", h=heads),
                            in1=exb[:, t * ew:(t + 1) * ew, None]
                                .broadcast_to([P, heads, cw]),
                            op=OP.mult)
                    else:
                        nc.vector.tensor_scalar_mul(v[:], g[:, t, :wd],
                                                    e_blk[:, t:t + 1])
                    nc.tensor.matmul(dp[:, :ew], lhsT=sem_s[:, t, :],
                                     rhs=exb[:, t * ew:(t + 1) * ew],
                                     start=(t == 0), stop=(t == T - 1))
                    for fc in range(nfc):
                        fsl = slice(fc * fcw, (fc + 1) * fcw)
                        nc.tensor.matmul(op_[:, fsl], lhsT=sem_s[:, t, :],
                                         rhs=v[:, fsl],
                                         start=(t == 0), stop=(t == T - 1))
                den = ep.tile([P, ew], F32, tag="den")
                nc.vector.tensor_tensor(
                    out=den[:], in0=dp[:, :ew],
                    in1=padv_s[:, b:b + 1].broadcast_to([P, ew]), op=OP.add)
                rden = ep.tile([P, ew], F32, tag="rden")
                nc.vector.reciprocal(rden[:], den[:])
                osb = ep.tile([P, wd], F32, tag="osb")
                if l < 2:
                    nc.vector.tensor_tensor(
                        out=osb[:].rearrange("p (h c) -> p h c", h=heads),
                        in0=op_[:].rearrange("p (h c) -> p h c", h=heads),
                        in1=rden[:, :, None].broadcast_to([P, heads, cw]),
                        op=OP.mult)
                else:
                    nc.vector.tensor_scalar_mul(osb[:], op_[:], rden[:, 0:1])
                # elu(+1) and residual
                mi = ep.tile([P, wd], F32, tag="mi")
                nc.vector.tensor_scalar_min(mi[:], osb[:], 0.0)
                u = ep.tile([P, wd], F32, tag="u")
                nc.scalar.activation(u[:], mi[:], AF.Exp)
                t0 = ep.tile([P, wd], F32, tag="t0")
                nc.vector.tensor_scalar_max(t0[:], osb[:], 0.0)
                nc.vector.tensor_add(t0[:], t0[:], u[:])
                hres = ep.tile([P, wd], F32, tag="hres")
                if l < 2:
                    nc.sync.dma_start(hres[:], h_i[l][bsl, :])
                else:
                    nc.sync.dma_start(hres[:], res2_i[bsl, :])
                nc.vector.tensor_add(t0[:], t0[:], hres[:])
                # LayerNorm
                nsg = max(1, wd // 512)
                sgw = min(wd, 512)
                stats = ep.tile([P, nsg, 6], F32, tag="stats")
                for sg in range(nsg):
                    nc.vector.bn_stats(stats[:, sg, :],
                                       t0[:, sg * sgw:(sg + 1) * sgw])
                mv = ep.tile([P, 2], F32, tag="mv")
                nc.vector.bn_aggr(mv[:], stats[:])
                q = ep.tile([P, 1], F32, tag="q")
                nc.scalar.activation(q[:], mv[:, 1:2], AF.Ln, bias=eps_s[:])
                rstd = ep.tile([P, 1], F32, tag="rstd")
                nc.scalar.activation(rstd[:], q[:], AF.Exp, scale=-0.5)
                nmr = ep.tile([P, 1], F32, tag="nmr")
                nc.vector.tensor_scalar(nmr[:], mv[:, 0:1], rstd[:, 0:1], -1.0,
                                        OP.mult, OP.mult)
                hn = ep.tile([P, wd], F32, tag="hn")
                nc.scalar.activation(hn[:], t0[:], AF.Identity,
                                     bias=nmr[:], scale=rstd[:, 0:1])
                if not wc[f"gtriv{l}"]:
                    nc.vector.tensor_mul(hn[:], hn[:], g_s[:])
                    nc.vector.tensor_add(hn[:], hn[:], be_s[:])
                rows = min(P, sh - b * P)
                if l < 2:
                    nc.sync.dma_start(h_i[l + 1][b * P:b * P + rows, :],
                                      hn[:rows, :])
                    hnb = ep.tile([P, wd], BF16, tag="hnb")
                    nc.vector.tensor_copy(hnb[:], hn[:])
                    nc.sync.dma_start(hb_i[l + 1][b * P:b * P + rows, :],
                                      hnb[:rows, :])
                else:
                    nc.sync.dma_start(out_d[b * P:b * P + rows, :], hn[:rows, :])
        if _stopped(f"e{l}"):
            return


# ------------------------------------------------------------------ run glue
_TRACE = {"on": False, "res": None}


def _run(inputs, n, din, h, c, out_w, sim=False):
    wc, cores = _prep(inputs, n, din, h, c, out_w)
    nc = _build(wc, n, din, h, c, out_w)
    shared = {k: v for k, v in wc.items()
              if isinstance(v, np.ndarray)}
    # drop replicated-affine tensors when trivial (not declared)
    for l in range(3):
        if wc[f"gtriv{l}"]:
            shared.pop(f"grep{l}", None)
            shared.pop(f"berep{l}", None)
    in_maps = [dict(shared, **cores[k]) for k in range(NCORES)]
    if sim:
        from concourse.bass_interp import MultiCoreSim
        ms = MultiCoreSim(nc, NCORES)
        for k in range(NCORES):
            for name, arr in in_maps[k].items():
                ms.cores[k].tensor(name)[:] = arr
        ms.simulate()
        outs = [np.array(ms.cores[k].tensor("out")[:]) for k in range(NCORES)]
        return np.concatenate(outs, 0)
    res = run_bass_kernel_spmd(nc, in_maps, list(range(NCORES)),
                               trace=_TRACE["on"])
    _TRACE["res"] = res
    return np.concatenate([res.results[k]["out"] for k in range(NCORES)], 0)


def kernel(**inputs):
    x = inputs["x"]
    n, din = x.shape
    h, c = inputs["att0"].shape
    out_w = inputs["att2"].shape[1]
    return np.asarray(_run(inputs, n, din, h, c, out_w), np.float32)
